# revision 6
# baseline (speedup 1.0000x reference)
"""Trainium2 Bass kernel for nn_Decoder (ragged per-layer decoder + norms/tanh/sparsity).

Self-contained: takes FULL inputs, shards d_features 8-ways across NeuronCores,
runs one SPMD Bass/Tile kernel, and reduces partial outputs on host.

reference semantics (B=32, L=12, D=6144, A=768, C=0.1, LAMBDA=0.1):
  recons[b, lp, a]   = sum_{l<=lp, d} feat[b, l, d] * W_lp[l, d, a]
  dec_norms[l, d]    = sum_{lp>=l} ||W_lp[l, d, :]||_2
  tanh               = tanh(feat * dec_norms * C)
  sparsity           = LAMBDA * sum_d mean_b sum_l tanh

Sharding: d-axis split 8 ways (768 per core). Each core computes partial
recons (summed over its d-shard; host adds the 8 partials) and a partial
tanh-sum vector (host finishes the scalar). Weights are cast to fp16 on host
(halves the HBM traffic, which is the roofline: weights are ~1.5 GB used
exactly once); matmul accumulates in fp32 PSUM. Norms are computed in fp32
from the fp16 tiles.
"""

import sys

if "/opt/trn_rl_repo" not in sys.path:
    sys.path.insert(0, "/opt/trn_rl_repo")

import numpy as np

B = 32
L = 12
DFULL = 6144
DA = 768
NCORES = 8
DL = DFULL // NCORES  # 768 d-features per core
DC = DL // 128  # 6 chunks of 128 partitions
P = 128
C = 0.1
LAMBDA = 0.1

W_BUFS = 12  # weight-slab prefetch depth ([128, 6, 768] fp16 = 9 KiB/partition each)
# norm-op engine split: DVE gets DVE_NUM out of every SPLIT_MOD tiles, ACT the rest
SPLIT_MOD, DVE_NUM = 13, 7


def _split_multi_waits(nc, maxw: int = 1):
    """This container's walrus rejects more than one semaphore wait per
    instruction ("Too many sync wait commands"), while Tile freely attaches
    several. Move excess waits onto preceding same-engine NOPs."""
    from concourse import mybir

    ctr = 0
    for bb in nc.main_func.blocks:
        out = []
        changed = False
        for ins in bb.instructions:
            si = getattr(ins, "sync_info", None)
            waits = list(si.on_wait) if si is not None and si.on_wait else []
            if len(waits) > maxw:
                changed = True
                extra, keep = waits[:-maxw], waits[-maxw:]
                for i in range(0, len(extra), maxw):
                    ctr += 1
                    nop = mybir.InstNoOp(name=f"waitsplit-{ctr}", ins=[], outs=[])
                    nop.engine = ins.engine
                    nop.sync_info = mybir.SyncInfo(
                        on_wait=extra[i : i + maxw], on_update=[]
                    )
                    out.append(nop)
                ins.sync_info = mybir.SyncInfo(
                    on_wait=keep, on_update=list(si.on_update or [])
                )
            out.append(ins)
        if changed:
            bb.instructions = out


def build_nc():
    """Build the per-core Bass module (same program on all 8 cores)."""
    import concourse.bass as bass
    import concourse.tile as tile
    from concourse import mybir

    f16 = mybir.dt.float16
    f32 = mybir.dt.float32
    bf16 = mybir.dt.bfloat16

    nc = bass.Bass(trn_type="TRN2")

    featT_h = nc.dram_tensor("featT", [P, L * DC, B], f16, kind="ExternalInput")
    W_h = [
        nc.dram_tensor(f"W_{lp}", [(lp + 1) * DL, DA], f16, kind="ExternalInput")
        for lp in range(L)
    ]
    recons_h = nc.dram_tensor("recons", [B, L, DA], f32, kind="ExternalOutput")
    tanh_part_h = nc.dram_tensor("tanh_part", [P, 1], f32, kind="ExternalOutput")

    # DRAM view of W_lp with row = l*768 + dc*128 + p  ->  [l, p, dc, a]
    W_ap = [
        W_h[lp][:].rearrange("(l dc p) a -> l p dc a", dc=DC, p=P) for lp in range(L)
    ]

    from contextlib import ExitStack

    with tile.TileContext(nc) as tc, ExitStack() as ctx:
        wpool = ctx.enter_context(tc.tile_pool(name="w", bufs=W_BUFS))
        const = ctx.enter_context(tc.tile_pool(name="const", bufs=1))
        sqpool = ctx.enter_context(tc.tile_pool(name="sq", bufs=2))
        rpool = ctx.enter_context(tc.tile_pool(name="r", bufs=4))
        psum = ctx.enter_context(tc.tile_pool(name="ps", bufs=3, space="PSUM"))
        outp = ctx.enter_context(tc.tile_pool(name="outp", bufs=3))

        ftile = const.tile([P, L * DC, B], f16, tag="ftile", name="ftile")
        nc.sync.dma_start(ftile[:], featT_h[:])

        # per-source-layer norm-square accumulators, one per engine so DVE-
        # and ACT-issued norm ops never touch the same tile. Column = dc*12+lp.
        norm_d = [const.tile([P, DC * L], f32, tag=f"nd{l}", name=f"nd{l}") for l in range(L)]
        norm_a = [const.tile([P, DC * L], f32, tag=f"na{l}", name=f"na{l}") for l in range(L)]
        for t in norm_d + norm_a:
            nc.gpsimd.memset(t[:], 0.0)

        tanh_sums = const.tile([P, L * DC], f32, tag="tanhsums", name="tanhsums")
        sq_d = const.tile([P, DA], bf16, tag="sqd", name="sqd")  # DVE square scratch (never read)
        sq_a = const.tile([P, DA], bf16, tag="sqa", name="sqa")  # ACT square scratch (never read)
        tanh_scr = const.tile([P, B], bf16, tag="tanhscr", name="tanhscr")  # tanh scratch (never read)
        dec = [const.tile([P, DC], f32, tag=f"dec{l}", name=f"dec{l}") for l in range(L)]

        norm_ctr = 0

        def emit_norm(wtile, l, dc, lp):
            nonlocal norm_ctr
            col = dc * L + lp
            use_dve = (norm_ctr % SPLIT_MOD) < DVE_NUM
            norm_ctr += 1
            wsl = wtile[:, dc : dc + 1, :]
            if use_dve:
                # out = (w * 1.0) * w ; accum_out = row-sum(out) = ||row||^2
                nc.vector.scalar_tensor_tensor(
                    out=sq_d[:],
                    in0=wsl,
                    scalar=1.0,
                    in1=wsl,
                    op0=mybir.AluOpType.mult,
                    op1=mybir.AluOpType.mult,
                    accum_out=norm_d[l][:, col : col + 1],
                )
            else:
                nc.scalar.activation(
                    sq_a[:],
                    wsl,
                    mybir.ActivationFunctionType.Square,
                    accum_out=norm_a[l][:, col : col + 1],
                )

        def finalize_l(l):
            """dec[l][:, dc] = C * sum_{lp} sqrt(norm_sq[l, dc, lp]); then tanh."""
            s_d = sqpool.tile([P, DC * L], f32, tag="s3", name="s3")
            s_a = sqpool.tile([P, DC * L], f32, tag="s3", name="s3")
            # sqrt(x * C^2) = C * sqrt(x)
            nc.scalar.activation(
                s_d[:], norm_d[l][:], mybir.ActivationFunctionType.Sqrt, scale=C * C
            )
            nc.scalar.activation(
                s_a[:], norm_a[l][:], mybir.ActivationFunctionType.Sqrt, scale=C * C
            )
            r_d = rpool.tile([P, DC], f32, tag="r", name="r")
            r_a = rpool.tile([P, DC], f32, tag="r", name="r")
            nc.vector.tensor_reduce(
                r_d[:],
                s_d[:].rearrange("p (dc lp) -> p dc lp", lp=L),
                axis=mybir.AxisListType.X,
                op=mybir.AluOpType.add,
            )
            nc.vector.tensor_reduce(
                r_a[:],
                s_a[:].rearrange("p (dc lp) -> p dc lp", lp=L),
                axis=mybir.AxisListType.X,
                op=mybir.AluOpType.add,
            )
            nc.vector.tensor_add(dec[l][:], r_d[:], r_a[:])
            for dc in range(DC):
                j = l * DC + dc
                nc.scalar.activation(
                    tanh_scr[:],
                    ftile[:, j : j + 1, :],
                    mybir.ActivationFunctionType.Tanh,
                    scale=dec[l][:, dc : dc + 1],
                    accum_out=tanh_sums[:, j : j + 1],
                )

        for lp in range(L):
            ps = psum.tile([B, DA], f32, tag="ps", name="ps")
            for l in range(lp + 1):
                w = wpool.tile([P, DC, DA], f16, tag="w", name="w")
                nc.sync.dma_start(w[:], W_ap[lp][l])
                for dc in range(DC):
                    lhsT = ftile[:, l * DC + dc : l * DC + dc + 1, :]
                    start = l == 0 and dc == 0
                    stop = l == lp and dc == DC - 1
                    nc.tensor.matmul(
                        ps[:, 0:512],
                        lhsT,
                        w[:, dc : dc + 1, 0:512],
                        start=start,
                        stop=stop,
                    )
                    nc.tensor.matmul(
                        ps[:, 512:DA],
                        lhsT,
                        w[:, dc : dc + 1, 512:DA],
                        start=start,
                        stop=stop,
                    )
                    emit_norm(w, l, dc, lp)
                if lp == L - 1:
                    finalize_l(l)
            stage = outp.tile([B, DA], f32, tag="stage", name="stage")
            nc.vector.tensor_copy(stage[:], ps[:])
            nc.sync.dma_start(recons_h[:, lp : lp + 1, :], stage[:])

        tanh_tot = const.tile([P, 1], f32, tag="tanhtot", name="tanhtot")
        nc.vector.tensor_reduce(
            tanh_tot[:],
            tanh_sums[:],
            axis=mybir.AxisListType.X,
            op=mybir.AluOpType.add,
        )
        nc.sync.dma_start(tanh_part_h[:], tanh_tot[:])

    _split_multi_waits(nc)
    return nc


def prep_inputs(inputs):
    """Shard + cast + transpose the full inputs into per-core input maps."""
    feat = np.ascontiguousarray(np.asarray(inputs["features"]), dtype=np.float32)
    # d = core*768 + dc*128 + p
    ft = feat.reshape(B, L, NCORES, DC, P).transpose(2, 4, 1, 3, 0)  # core,p,l,dc,b
    ft = np.ascontiguousarray(ft).astype(np.float16).reshape(NCORES, P, L * DC, B)

    in_maps = [{"featT": ft[i]} for i in range(NCORES)]
    for lp in range(L):
        Wl = np.asarray(inputs[f"W_{lp}"])  # [lp+1, 6144, 768]
        Ws = (
            Wl.reshape(lp + 1, NCORES, DL, DA)
            .transpose(1, 0, 2, 3)
            .astype(np.float16)
            .reshape(NCORES, (lp + 1) * DL, DA)
        )
        Ws = np.ascontiguousarray(Ws)
        for i in range(NCORES):
            in_maps[i][f"W_{lp}"] = Ws[i]
    return in_maps


_NC_CACHE = {}


def get_nc():
    if "nc" not in _NC_CACHE:
        _NC_CACHE["nc"] = build_nc()
    return _NC_CACHE["nc"]


def run(inputs, trace=False, **kw):
    from concourse.bass_utils import run_bass_kernel_spmd

    nc = get_nc()
    in_maps = prep_inputs(inputs)
    res = run_bass_kernel_spmd(
        nc, in_maps, core_ids=list(range(NCORES)), trace=trace, **kw
    )
    recons = np.sum(
        np.stack([r["recons"] for r in res.results]), axis=0, dtype=np.float64
    ).astype(np.float32)
    tanh_total = float(
        np.sum([np.sum(r["tanh_part"], dtype=np.float64) for r in res.results])
    )
    sparsity = np.float32(LAMBDA / B * tanh_total)
    return (recons, sparsity), res


def kernel(**inputs):
    out, _ = run(inputs, trace=False)
    return out


# revision 9
# speedup vs baseline: 1.0768x; 1.0768x over previous
"""Trainium2 Bass kernel for nn_Decoder (ragged per-layer decoder + norms/tanh/sparsity).

Self-contained: takes FULL inputs, shards d_features 8-ways across NeuronCores,
runs one SPMD Bass/Tile kernel, and reduces partial outputs on host.

reference semantics (B=32, L=12, D=6144, A=768, C=0.1, LAMBDA=0.1):
  recons[b, lp, a]   = sum_{l<=lp, d} feat[b, l, d] * W_lp[l, d, a]
  dec_norms[l, d]    = sum_{lp>=l} ||W_lp[l, d, :]||_2
  tanh               = tanh(feat * dec_norms * C)
  sparsity           = LAMBDA * sum_d mean_b sum_l tanh

Sharding: d-axis split 8 ways (768 per core). Each core computes partial
recons (summed over its d-shard; host adds the 8 partials) and a partial
tanh-sum vector (host finishes the scalar). Weights are cast to fp16 on host
(halves the HBM traffic, which is the roofline: weights are ~1.5 GB used
exactly once); matmul accumulates in fp32 PSUM. Norms are computed in fp32
from the fp16 tiles.
"""

import sys

if "/opt/trn_rl_repo" not in sys.path:
    sys.path.insert(0, "/opt/trn_rl_repo")

import numpy as np

B = 32
L = 12
DFULL = 6144
DA = 768
NCORES = 8
DL = DFULL // NCORES  # 768 d-features per core
DC = DL // 128  # 6 chunks of 128 partitions
P = 128
C = 0.1
LAMBDA = 0.1

W_BUFS = 16  # weight-slab prefetch depth ([128, 6, 768] fp16 = 9 KiB/partition each)
# norm-op engine split pattern across DVE / ACT / GPSIMD (by per-tile op cost)
NORM_PATTERN = ("v", "a")  # alternate DVE / ACT (GPSIMD can't run STT in this walrus)
SCALE_SQ = 1024.0  # squares scaled so the fp16 scratch stays in normal range


def _split_multi_waits(nc, maxw: int = 1):
    """This container's walrus rejects more than one semaphore wait per
    instruction ("Too many sync wait commands"), while Tile freely attaches
    several. Move excess waits onto preceding same-engine NOPs."""
    from concourse import mybir

    ctr = 0
    for bb in nc.main_func.blocks:
        out = []
        changed = False
        for ins in bb.instructions:
            si = getattr(ins, "sync_info", None)
            waits = list(si.on_wait) if si is not None and si.on_wait else []
            if len(waits) > maxw:
                changed = True
                extra, keep = waits[:-maxw], waits[-maxw:]
                for i in range(0, len(extra), maxw):
                    ctr += 1
                    nop = mybir.InstNoOp(name=f"waitsplit-{ctr}", ins=[], outs=[])
                    nop.engine = ins.engine
                    nop.sync_info = mybir.SyncInfo(
                        on_wait=extra[i : i + maxw], on_update=[]
                    )
                    out.append(nop)
                ins.sync_info = mybir.SyncInfo(
                    on_wait=keep, on_update=list(si.on_update or [])
                )
            out.append(ins)
        if changed:
            bb.instructions = out


def build_nc():
    """Build the per-core Bass module (same program on all 8 cores)."""
    import concourse.bass as bass
    import concourse.tile as tile
    from concourse import mybir

    f16 = mybir.dt.float16
    f32 = mybir.dt.float32
    bf16 = mybir.dt.bfloat16

    nc = bass.Bass(trn_type="TRN2")

    featT_h = nc.dram_tensor("featT", [P, L * DC, B], f16, kind="ExternalInput")
    W_h = [
        nc.dram_tensor(f"W_{lp}", [(lp + 1) * DL, DA], f16, kind="ExternalInput")
        for lp in range(L)
    ]
    recons_h = nc.dram_tensor("recons", [B, L, DA], f32, kind="ExternalOutput")
    tanh_part_h = nc.dram_tensor("tanh_part", [P, 1], f32, kind="ExternalOutput")

    # DRAM view of W_lp with row = l*768 + dc*128 + p  ->  [l, p, dc, a]
    W_ap = [
        W_h[lp][:].rearrange("(l dc p) a -> l p dc a", dc=DC, p=P) for lp in range(L)
    ]

    from contextlib import ExitStack

    with tile.TileContext(nc) as tc, ExitStack() as ctx:
        wpool = ctx.enter_context(tc.tile_pool(name="w", bufs=W_BUFS))
        const = ctx.enter_context(tc.tile_pool(name="const", bufs=1))
        sqpool = ctx.enter_context(tc.tile_pool(name="sq", bufs=6))
        rpool = ctx.enter_context(tc.tile_pool(name="r", bufs=6))
        psum = ctx.enter_context(tc.tile_pool(name="ps", bufs=3, space="PSUM"))
        outp = ctx.enter_context(tc.tile_pool(name="outp", bufs=3))

        ftile = const.tile([P, L * DC, B], f16, tag="ftile", name="ftile")
        nc.sync.dma_start(ftile[:], featT_h[:])

        # per-source-layer norm-square accumulators, one per engine so DVE-
        # and ACT-issued norm ops never touch the same tile. Column = dc*12+lp.
        norm_d = [const.tile([P, DC * L], f32, tag=f"nd{l}", name=f"nd{l}") for l in range(L)]
        norm_a = [const.tile([P, DC * L], f32, tag=f"na{l}", name=f"na{l}") for l in range(L)]
        for t in norm_d + norm_a:
            nc.gpsimd.memset(t[:], 0.0)

        tanh_out = const.tile([P, L * DC, B], f32, tag="tanhout", name="tanhout")
        sq_d = const.tile([P, DA], f16, tag="sqd", name="sqd")  # DVE square scratch (never read)
        sq_a = const.tile([P, DA], f16, tag="sqa", name="sqa")  # ACT square scratch (never read)
        dec = [const.tile([P, DC], f32, tag=f"dec{l}", name=f"dec{l}") for l in range(L)]

        norm_ctr = 0

        def emit_norm(wtile, l, dc, lp):
            nonlocal norm_ctr
            col = dc * L + lp
            eng = NORM_PATTERN[norm_ctr % len(NORM_PATTERN)]
            norm_ctr += 1
            wsl = wtile[:, dc : dc + 1, :]
            if eng == "a":
                # out = (w*32)^2 = SCALE_SQ*w^2 ; accum_out = row-sum
                nc.scalar.activation(
                    sq_a[:],
                    wsl,
                    mybir.ActivationFunctionType.Square,
                    scale=float(np.sqrt(SCALE_SQ)),
                    accum_out=norm_a[l][:, col : col + 1],
                )
            else:
                # out = (w * SCALE_SQ) * w ; accum_out = row-sum = SCALE_SQ*||row||^2
                nc.vector.scalar_tensor_tensor(
                    out=sq_d[:],
                    in0=wsl,
                    scalar=SCALE_SQ,
                    in1=wsl,
                    op0=mybir.AluOpType.mult,
                    op1=mybir.AluOpType.mult,
                    accum_out=norm_d[l][:, col : col + 1],
                )

        def finalize_chunk(ls):
            """For each l in ls: dec[l] = C * sum_lp sqrt(norm_sq); then the
            tanh tiles. Batched so the ACT function table switches only twice
            per chunk (Sqrt once, Tanh once) instead of per-l."""
            rs = {}
            for l in ls:
                s_d = sqpool.tile([P, DC * L], f32, tag="s3", name="s3")
                s_a = sqpool.tile([P, DC * L], f32, tag="s3", name="s3")
                # sqrt(x * C^2 / SCALE_SQ) = C * sqrt(||row||^2)
                for srcb, dst in ((norm_d[l], s_d), (norm_a[l], s_a)):
                    nc.scalar.activation(
                        dst[:],
                        srcb[:],
                        mybir.ActivationFunctionType.Sqrt,
                        scale=C * C / SCALE_SQ,
                    )
                rs[l] = (s_d, s_a)
            for l in ls:
                s_d, s_a = rs[l]
                r_d = rpool.tile([P, DC], f32, tag="r", name="r")
                r_a = rpool.tile([P, DC], f32, tag="r", name="r")
                for s, r in ((s_d, r_d), (s_a, r_a)):
                    nc.vector.tensor_reduce(
                        r[:],
                        s[:].rearrange("p (dc lp) -> p dc lp", lp=L),
                        axis=mybir.AxisListType.X,
                        op=mybir.AluOpType.add,
                    )
                nc.vector.tensor_add(dec[l][:], r_d[:], r_a[:])
            for l in ls:
                for dc in range(DC):
                    j = l * DC + dc
                    nc.scalar.activation(
                        tanh_out[:, j : j + 1, :],
                        ftile[:, j : j + 1, :],
                        mybir.ActivationFunctionType.Tanh,
                        scale=dec[l][:, dc : dc + 1],
                    )

        for lp in range(L):
            ps = psum.tile([B, DA], f32, tag="ps", name="ps")
            for l in range(lp + 1):
                w = wpool.tile([P, DC, DA], f16, tag="w", name="w")
                nc.sync.dma_start(w[:], W_ap[lp][l])
                for dc in range(DC):
                    lhsT = ftile[:, l * DC + dc : l * DC + dc + 1, :]
                    start = l == 0 and dc == 0
                    stop = l == lp and dc == DC - 1
                    nc.tensor.matmul(
                        ps[:, 0:512],
                        lhsT,
                        w[:, dc : dc + 1, 0:512],
                        start=start,
                        stop=stop,
                    )
                    nc.tensor.matmul(
                        ps[:, 512:DA],
                        lhsT,
                        w[:, dc : dc + 1, 512:DA],
                        start=start,
                        stop=stop,
                    )
                    emit_norm(w, l, dc, lp)
                if lp == L - 1 and l in (L // 2 - 1, L - 1):
                    finalize_chunk(
                        range(0, L // 2) if l == L // 2 - 1 else range(L // 2, L)
                    )
            stage = outp.tile([B, DA], f32, tag="stage", name="stage")
            nc.vector.tensor_copy(stage[:], ps[:])
            nc.sync.dma_start(recons_h[:, lp : lp + 1, :], stage[:])

        tanh_tot = const.tile([P, 1], f32, tag="tanhtot", name="tanhtot")
        nc.vector.tensor_reduce(
            tanh_tot[:],
            tanh_out[:],
            axis=mybir.AxisListType.XY,
            op=mybir.AluOpType.add,
        )
        nc.sync.dma_start(tanh_part_h[:], tanh_tot[:])

    _split_multi_waits(nc)
    return nc


def prep_inputs(inputs):
    """Shard + cast + transpose the full inputs into per-core input maps."""
    feat = np.ascontiguousarray(np.asarray(inputs["features"]), dtype=np.float32)
    # d = core*768 + dc*128 + p
    ft = feat.reshape(B, L, NCORES, DC, P).transpose(2, 4, 1, 3, 0)  # core,p,l,dc,b
    ft = np.ascontiguousarray(ft).astype(np.float16).reshape(NCORES, P, L * DC, B)

    in_maps = [{"featT": ft[i]} for i in range(NCORES)]
    for lp in range(L):
        Wl = np.asarray(inputs[f"W_{lp}"])  # [lp+1, 6144, 768]
        Ws = (
            Wl.reshape(lp + 1, NCORES, DL, DA)
            .transpose(1, 0, 2, 3)
            .astype(np.float16)
            .reshape(NCORES, (lp + 1) * DL, DA)
        )
        Ws = np.ascontiguousarray(Ws)
        for i in range(NCORES):
            in_maps[i][f"W_{lp}"] = Ws[i]
    return in_maps


_NC_CACHE = {}


def get_nc():
    if "nc" not in _NC_CACHE:
        _NC_CACHE["nc"] = build_nc()
    return _NC_CACHE["nc"]


def run(inputs, trace=False, **kw):
    from concourse.bass_utils import run_bass_kernel_spmd

    nc = get_nc()
    in_maps = prep_inputs(inputs)
    res = run_bass_kernel_spmd(
        nc, in_maps, core_ids=list(range(NCORES)), trace=trace, **kw
    )
    recons = np.sum(
        np.stack([r["recons"] for r in res.results]), axis=0, dtype=np.float64
    ).astype(np.float32)
    tanh_total = float(
        np.sum([np.sum(r["tanh_part"], dtype=np.float64) for r in res.results])
    )
    sparsity = np.float32(LAMBDA / B * tanh_total)
    return (recons, sparsity), res


def kernel(**inputs):
    out, _ = run(inputs, trace=False)
    return out


# revision 10
# speedup vs baseline: 1.2071x; 1.1210x over previous
"""Trainium2 Bass kernel for nn_Decoder (ragged per-layer decoder + norms/tanh/sparsity).

Self-contained: takes FULL inputs, shards d_features 8-ways across NeuronCores,
runs one SPMD Bass/Tile kernel, and reduces partial outputs on host.

reference semantics (B=32, L=12, D=6144, A=768, C=0.1, LAMBDA=0.1):
  recons[b, lp, a]   = sum_{l<=lp, d} feat[b, l, d] * W_lp[l, d, a]
  dec_norms[l, d]    = sum_{lp>=l} ||W_lp[l, d, :]||_2
  tanh               = tanh(feat * dec_norms * C)
  sparsity           = LAMBDA * sum_d mean_b sum_l tanh

Sharding: d-axis split 8 ways (768 per core). Each core computes partial
recons (summed over its d-shard; host adds the 8 partials) and a partial
tanh-sum vector (host finishes the scalar). Weights are cast to fp16 on host
(halves the HBM traffic, which is the roofline: weights are ~1.5 GB used
exactly once); matmul accumulates in fp32 PSUM. Norms are computed in fp32
from the fp16 tiles.
"""

import sys

if "/opt/trn_rl_repo" not in sys.path:
    sys.path.insert(0, "/opt/trn_rl_repo")

import numpy as np

B = 32
L = 12
DFULL = 6144
DA = 768
NCORES = 8
DL = DFULL // NCORES  # 768 d-features per core
DC = DL // 128  # 6 chunks of 128 partitions
P = 128
C = 0.1
LAMBDA = 0.1

W_BUFS = 16  # weight-slab prefetch depth ([128, 6, 768] fp16 = 9 KiB/partition each)
# norm-op engine split pattern across DVE / ACT / GPSIMD (by per-tile op cost)
NORM_PATTERN = ("v", "a")  # alternate DVE / ACT (GPSIMD can't run STT in this walrus)
SCALE_SQ = 1024.0  # squares scaled so the fp16 scratch stays in normal range
W_DTYPE = "float16"  # "float16" (safer) or "bfloat16" (may unlock 2x DVE mode)


def _split_multi_waits(nc, maxw: int = 1):
    """This container's walrus rejects more than one semaphore wait per
    instruction ("Too many sync wait commands"), while Tile freely attaches
    several. Move excess waits onto preceding same-engine NOPs."""
    from concourse import mybir

    ctr = 0
    for bb in nc.main_func.blocks:
        out = []
        changed = False
        for ins in bb.instructions:
            si = getattr(ins, "sync_info", None)
            waits = list(si.on_wait) if si is not None and si.on_wait else []
            if len(waits) > maxw:
                changed = True
                extra, keep = waits[:-maxw], waits[-maxw:]
                for i in range(0, len(extra), maxw):
                    ctr += 1
                    nop = mybir.InstNoOp(name=f"waitsplit-{ctr}", ins=[], outs=[])
                    nop.engine = ins.engine
                    nop.sync_info = mybir.SyncInfo(
                        on_wait=extra[i : i + maxw], on_update=[]
                    )
                    out.append(nop)
                ins.sync_info = mybir.SyncInfo(
                    on_wait=keep, on_update=list(si.on_update or [])
                )
            out.append(ins)
        if changed:
            bb.instructions = out


def build_nc():
    """Build the per-core Bass module (same program on all 8 cores)."""
    import concourse.bass as bass
    import concourse.tile as tile
    from concourse import mybir

    f16 = getattr(mybir.dt, W_DTYPE)
    f32 = mybir.dt.float32
    bf16 = mybir.dt.bfloat16

    nc = bass.Bass(trn_type="TRN2")

    featT_h = nc.dram_tensor("featT", [P, L * DC, B], f16, kind="ExternalInput")
    W_h = [
        nc.dram_tensor(f"W_{lp}", [(lp + 1) * DL, DA], f16, kind="ExternalInput")
        for lp in range(L)
    ]
    recons_h = nc.dram_tensor("recons", [B, L, DA], f32, kind="ExternalOutput")
    tanh_part_h = nc.dram_tensor("tanh_part", [P, 1], f32, kind="ExternalOutput")

    # DRAM view of W_lp with row = l*768 + dc*128 + p  ->  [l, p, dc, a]
    W_ap = [
        W_h[lp][:].rearrange("(l dc p) a -> l p dc a", dc=DC, p=P) for lp in range(L)
    ]

    from contextlib import ExitStack

    with tile.TileContext(nc) as tc, ExitStack() as ctx:
        wpool = ctx.enter_context(tc.tile_pool(name="w", bufs=W_BUFS))
        const = ctx.enter_context(tc.tile_pool(name="const", bufs=1))
        sqpool = ctx.enter_context(tc.tile_pool(name="sq", bufs=6))
        rpool = ctx.enter_context(tc.tile_pool(name="r", bufs=6))
        psum = ctx.enter_context(tc.tile_pool(name="ps", bufs=3, space="PSUM"))
        outp = ctx.enter_context(tc.tile_pool(name="outp", bufs=3))

        ftile = const.tile([P, L * DC, B], f16, tag="ftile", name="ftile")
        nc.sync.dma_start(ftile[:], featT_h[:])

        # per-source-layer norm-square accumulators, one per engine so DVE-
        # and ACT-issued norm ops never touch the same tile. Column = dc*12+lp.
        norm_d = [const.tile([P, DC * L], f32, tag=f"nd{l}", name=f"nd{l}") for l in range(L)]
        norm_a = [const.tile([P, DC * L], f32, tag=f"na{l}", name=f"na{l}") for l in range(L)]
        for t in norm_d + norm_a:
            nc.gpsimd.memset(t[:], 0.0)

        tanh_out = const.tile([P, L * DC, B], f32, tag="tanhout", name="tanhout")
        sq_d = const.tile([P, DA], f16, tag="sqd", name="sqd")  # DVE square scratch (never read)
        sq_a = const.tile([P, DA], f16, tag="sqa", name="sqa")  # ACT square scratch (never read)
        dec = [const.tile([P, DC], f32, tag=f"dec{l}", name=f"dec{l}") for l in range(L)]

        norm_ctr = 0

        def emit_norm(wtile, l, dc, lp):
            nonlocal norm_ctr
            col = dc * L + lp
            eng = NORM_PATTERN[norm_ctr % len(NORM_PATTERN)]
            norm_ctr += 1
            if lp == L - 1 and l > 2:
                # keep ACT free for the finalize sqrt/tanh batches at the tail
                eng = "v"
            wsl = wtile[:, dc : dc + 1, :]
            if eng == "a":
                # out = (w*32)^2 = SCALE_SQ*w^2 ; accum_out = row-sum
                nc.scalar.activation(
                    sq_a[:],
                    wsl,
                    mybir.ActivationFunctionType.Square,
                    scale=float(np.sqrt(SCALE_SQ)),
                    accum_out=norm_a[l][:, col : col + 1],
                )
            else:
                # out = (w * SCALE_SQ) * w ; accum_out = row-sum = SCALE_SQ*||row||^2
                nc.vector.scalar_tensor_tensor(
                    out=sq_d[:],
                    in0=wsl,
                    scalar=SCALE_SQ,
                    in1=wsl,
                    op0=mybir.AluOpType.mult,
                    op1=mybir.AluOpType.mult,
                    accum_out=norm_d[l][:, col : col + 1],
                )

        def finalize_chunk(ls):
            """For each l in ls: dec[l] = C * sum_lp sqrt(norm_sq); then the
            tanh tiles. Batched so the ACT function table switches only twice
            per chunk (Sqrt once, Tanh once) instead of per-l."""
            rs = {}
            for l in ls:
                s_d = sqpool.tile([P, DC * L], f32, tag="s3", name="s3")
                s_a = sqpool.tile([P, DC * L], f32, tag="s3", name="s3")
                # sqrt(x * C^2 / SCALE_SQ) = C * sqrt(||row||^2)
                for srcb, dst in ((norm_d[l], s_d), (norm_a[l], s_a)):
                    nc.scalar.activation(
                        dst[:],
                        srcb[:],
                        mybir.ActivationFunctionType.Sqrt,
                        scale=C * C / SCALE_SQ,
                    )
                rs[l] = (s_d, s_a)
            for l in ls:
                s_d, s_a = rs[l]
                r_d = rpool.tile([P, DC], f32, tag="r", name="r")
                r_a = rpool.tile([P, DC], f32, tag="r", name="r")
                for s, r in ((s_d, r_d), (s_a, r_a)):
                    nc.vector.tensor_reduce(
                        r[:],
                        s[:].rearrange("p (dc lp) -> p dc lp", lp=L),
                        axis=mybir.AxisListType.X,
                        op=mybir.AluOpType.add,
                    )
                nc.vector.tensor_add(dec[l][:], r_d[:], r_a[:])
            for l in ls:
                for dc in range(DC):
                    j = l * DC + dc
                    nc.scalar.activation(
                        tanh_out[:, j : j + 1, :],
                        ftile[:, j : j + 1, :],
                        mybir.ActivationFunctionType.Tanh,
                        scale=dec[l][:, dc : dc + 1],
                    )

        for lp in range(L):
            ps = psum.tile([B, DA], f32, tag="ps", name="ps")
            for l in range(lp + 1):
                w = wpool.tile([P, DC, DA], f16, tag="w", name="w")
                nc.sync.dma_start(w[:], W_ap[lp][l])
                for dc in range(DC):
                    lhsT = ftile[:, l * DC + dc : l * DC + dc + 1, :]
                    start = l == 0 and dc == 0
                    stop = l == lp and dc == DC - 1
                    nc.tensor.matmul(
                        ps[:, 0:512],
                        lhsT,
                        w[:, dc : dc + 1, 0:512],
                        start=start,
                        stop=stop,
                    )
                    nc.tensor.matmul(
                        ps[:, 512:DA],
                        lhsT,
                        w[:, dc : dc + 1, 512:DA],
                        start=start,
                        stop=stop,
                    )
                    emit_norm(w, l, dc, lp)
                if lp == L - 1 and l in (L // 2 - 1, L - 1):
                    finalize_chunk(
                        range(0, L // 2) if l == L // 2 - 1 else range(L // 2, L)
                    )
            stage = outp.tile([B, DA], f32, tag="stage", name="stage")
            nc.vector.tensor_copy(stage[:], ps[:])
            nc.sync.dma_start(recons_h[:, lp : lp + 1, :], stage[:])

        tanh_tot = const.tile([P, 1], f32, tag="tanhtot", name="tanhtot")
        nc.vector.tensor_reduce(
            tanh_tot[:],
            tanh_out[:],
            axis=mybir.AxisListType.XY,
            op=mybir.AluOpType.add,
        )
        nc.sync.dma_start(tanh_part_h[:], tanh_tot[:])

    _split_multi_waits(nc)
    return nc


def prep_inputs(inputs):
    """Shard + cast + transpose the full inputs into per-core input maps."""
    import ml_dtypes

    wdt = np.float16 if W_DTYPE == "float16" else ml_dtypes.bfloat16
    feat = np.ascontiguousarray(np.asarray(inputs["features"]), dtype=np.float32)
    # d = core*768 + dc*128 + p
    ft = feat.reshape(B, L, NCORES, DC, P).transpose(2, 4, 1, 3, 0)  # core,p,l,dc,b
    ft = np.ascontiguousarray(ft).astype(wdt).reshape(NCORES, P, L * DC, B)

    in_maps = [{"featT": ft[i]} for i in range(NCORES)]
    for lp in range(L):
        Wl = np.asarray(inputs[f"W_{lp}"])  # [lp+1, 6144, 768]
        Ws = (
            Wl.reshape(lp + 1, NCORES, DL, DA)
            .transpose(1, 0, 2, 3)
            .astype(wdt)
            .reshape(NCORES, (lp + 1) * DL, DA)
        )
        Ws = np.ascontiguousarray(Ws)
        for i in range(NCORES):
            in_maps[i][f"W_{lp}"] = Ws[i]
    return in_maps


_NC_CACHE = {}


def get_nc():
    if "nc" not in _NC_CACHE:
        _NC_CACHE["nc"] = build_nc()
    return _NC_CACHE["nc"]


def run(inputs, trace=False, **kw):
    from concourse.bass_utils import run_bass_kernel_spmd

    nc = get_nc()
    in_maps = prep_inputs(inputs)
    res = run_bass_kernel_spmd(
        nc, in_maps, core_ids=list(range(NCORES)), trace=trace, **kw
    )
    recons = np.sum(
        np.stack([r["recons"] for r in res.results]), axis=0, dtype=np.float64
    ).astype(np.float32)
    tanh_total = float(
        np.sum([np.sum(r["tanh_part"], dtype=np.float64) for r in res.results])
    )
    sparsity = np.float32(LAMBDA / B * tanh_total)
    return (recons, sparsity), res


def kernel(**inputs):
    out, _ = run(inputs, trace=False)
    return out


# revision 11
# speedup vs baseline: 1.2515x; 1.0367x over previous
"""Trainium2 Bass kernel for nn_Decoder (ragged per-layer decoder + norms/tanh/sparsity).

Self-contained: takes FULL inputs, shards d_features 8-ways across NeuronCores,
runs one SPMD Bass/Tile kernel, and reduces partial outputs on host.

reference semantics (B=32, L=12, D=6144, A=768, C=0.1, LAMBDA=0.1):
  recons[b, lp, a]   = sum_{l<=lp, d} feat[b, l, d] * W_lp[l, d, a]
  dec_norms[l, d]    = sum_{lp>=l} ||W_lp[l, d, :]||_2
  tanh               = tanh(feat * dec_norms * C)
  sparsity           = LAMBDA * sum_d mean_b sum_l tanh

Sharding: d-axis split 8 ways (768 per core). Each core computes partial
recons (summed over its d-shard; host adds the 8 partials) and a partial
tanh-sum vector (host finishes the scalar). Weights are cast to fp16 on host
(halves the HBM traffic, which is the roofline: weights are ~1.5 GB used
exactly once); matmul accumulates in fp32 PSUM. Norms are computed in fp32
from the fp16 tiles.
"""

import sys

if "/opt/trn_rl_repo" not in sys.path:
    sys.path.insert(0, "/opt/trn_rl_repo")

import numpy as np

B = 32
L = 12
DFULL = 6144
DA = 768
NCORES = 8
DL = DFULL // NCORES  # 768 d-features per core
DC = DL // 128  # 6 chunks of 128 partitions
P = 128
C = 0.1
LAMBDA = 0.1

W_BUFS = 16  # weight-slab prefetch depth ([128, 6, 768] fp16 = 9 KiB/partition each)
# norm-op engine split pattern across DVE / ACT / GPSIMD (by per-tile op cost)
NORM_PATTERN = ("v", "a")  # alternate DVE / ACT (GPSIMD can't run STT in this walrus)
SCALE_SQ = 1024.0  # squares scaled so the fp16 scratch stays in normal range
W_DTYPE = "bfloat16"  # "float16" (safer) or "bfloat16" (may unlock 2x DVE mode)


def _split_multi_waits(nc, maxw: int = 1):
    """This container's walrus rejects more than one semaphore wait per
    instruction ("Too many sync wait commands"), while Tile freely attaches
    several. Move excess waits onto preceding same-engine NOPs."""
    from concourse import mybir

    ctr = 0
    for bb in nc.main_func.blocks:
        out = []
        changed = False
        for ins in bb.instructions:
            si = getattr(ins, "sync_info", None)
            waits = list(si.on_wait) if si is not None and si.on_wait else []
            if len(waits) > maxw:
                changed = True
                extra, keep = waits[:-maxw], waits[-maxw:]
                for i in range(0, len(extra), maxw):
                    ctr += 1
                    nop = mybir.InstNoOp(name=f"waitsplit-{ctr}", ins=[], outs=[])
                    nop.engine = ins.engine
                    nop.sync_info = mybir.SyncInfo(
                        on_wait=extra[i : i + maxw], on_update=[]
                    )
                    out.append(nop)
                ins.sync_info = mybir.SyncInfo(
                    on_wait=keep, on_update=list(si.on_update or [])
                )
            out.append(ins)
        if changed:
            bb.instructions = out


def build_nc():
    """Build the per-core Bass module (same program on all 8 cores)."""
    import concourse.bass as bass
    import concourse.tile as tile
    from concourse import mybir

    f16 = getattr(mybir.dt, W_DTYPE)
    f32 = mybir.dt.float32
    bf16 = mybir.dt.bfloat16

    nc = bass.Bass(trn_type="TRN2")

    featT_h = nc.dram_tensor("featT", [P, L * DC, B], f16, kind="ExternalInput")
    W_h = [
        nc.dram_tensor(f"W_{lp}", [(lp + 1) * DL, DA], f16, kind="ExternalInput")
        for lp in range(L)
    ]
    recons_h = nc.dram_tensor("recons", [B, L, DA], f32, kind="ExternalOutput")
    tanh_part_h = nc.dram_tensor("tanh_part", [P, 1], f32, kind="ExternalOutput")

    # DRAM view of W_lp with row = l*768 + dc*128 + p  ->  [l, p, dc, a]
    W_ap = [
        W_h[lp][:].rearrange("(l dc p) a -> l p dc a", dc=DC, p=P) for lp in range(L)
    ]

    from contextlib import ExitStack

    with tile.TileContext(nc) as tc, ExitStack() as ctx:
        wpool = ctx.enter_context(tc.tile_pool(name="w", bufs=W_BUFS))
        const = ctx.enter_context(tc.tile_pool(name="const", bufs=1))
        sqpool = ctx.enter_context(tc.tile_pool(name="sq", bufs=6))
        rpool = ctx.enter_context(tc.tile_pool(name="r", bufs=6))
        psum = ctx.enter_context(tc.tile_pool(name="ps", bufs=3, space="PSUM"))
        outp = ctx.enter_context(tc.tile_pool(name="outp", bufs=3))

        ftile = const.tile([P, L * DC, B], f16, tag="ftile", name="ftile")
        nc.sync.dma_start(ftile[:], featT_h[:])

        # per-source-layer norm-square accumulators, one per engine so DVE-
        # and ACT-issued norm ops never touch the same tile. Column = dc*12+lp.
        norm_d = [const.tile([P, DC * L], f32, tag=f"nd{l}", name=f"nd{l}") for l in range(L)]
        norm_a = [const.tile([P, DC * L], f32, tag=f"na{l}", name=f"na{l}") for l in range(L)]
        for t in norm_d + norm_a:
            nc.gpsimd.memset(t[:], 0.0)

        tanh_out = const.tile([P, L * DC, B], f32, tag="tanhout", name="tanhout")
        sq_d = const.tile([P, DA], f16, tag="sqd", name="sqd")  # DVE square scratch (never read)
        sq_a = const.tile([P, DA], f16, tag="sqa", name="sqa")  # ACT square scratch (never read)
        dec = [const.tile([P, DC], f32, tag=f"dec{l}", name=f"dec{l}") for l in range(L)]

        norm_ctr = 0

        def emit_norm(wtile, l, dc, lp):
            nonlocal norm_ctr
            col = dc * L + lp
            eng = NORM_PATTERN[norm_ctr % len(NORM_PATTERN)]
            norm_ctr += 1
            if lp == L - 1 and l > 2:
                # keep ACT free for the finalize sqrt/tanh batches at the tail
                eng = "v"
            wsl = wtile[:, dc : dc + 1, :]
            if eng == "a":
                # out = (w*32)^2 = SCALE_SQ*w^2 ; accum_out = row-sum
                nc.scalar.activation(
                    sq_a[:],
                    wsl,
                    mybir.ActivationFunctionType.Square,
                    scale=float(np.sqrt(SCALE_SQ)),
                    accum_out=norm_a[l][:, col : col + 1],
                )
            else:
                # out = (w * SCALE_SQ) * w ; accum_out = row-sum = SCALE_SQ*||row||^2
                nc.vector.scalar_tensor_tensor(
                    out=sq_d[:],
                    in0=wsl,
                    scalar=SCALE_SQ,
                    in1=wsl,
                    op0=mybir.AluOpType.mult,
                    op1=mybir.AluOpType.mult,
                    accum_out=norm_d[l][:, col : col + 1],
                )

        def finalize_chunk(ls):
            """For each l in ls: dec[l] = C * sum_lp sqrt(norm_sq); then the
            tanh tiles. Batched so the ACT function table switches only twice
            per chunk (Sqrt once, Tanh once) instead of per-l."""
            rs = {}
            for l in ls:
                s_d = sqpool.tile([P, DC * L], f32, tag="s3", name="s3")
                s_a = sqpool.tile([P, DC * L], f32, tag="s3", name="s3")
                # sqrt(x * C^2 / SCALE_SQ) = C * sqrt(||row||^2)
                for srcb, dst in ((norm_d[l], s_d), (norm_a[l], s_a)):
                    nc.scalar.activation(
                        dst[:],
                        srcb[:],
                        mybir.ActivationFunctionType.Sqrt,
                        scale=C * C / SCALE_SQ,
                    )
                rs[l] = (s_d, s_a)
            for l in ls:
                s_d, s_a = rs[l]
                r_d = rpool.tile([P, DC], f32, tag="r", name="r")
                r_a = rpool.tile([P, DC], f32, tag="r", name="r")
                for s, r in ((s_d, r_d), (s_a, r_a)):
                    nc.vector.tensor_reduce(
                        r[:],
                        s[:].rearrange("p (dc lp) -> p dc lp", lp=L),
                        axis=mybir.AxisListType.X,
                        op=mybir.AluOpType.add,
                    )
                nc.vector.tensor_add(dec[l][:], r_d[:], r_a[:])
            for l in ls:
                for dc in range(DC):
                    j = l * DC + dc
                    nc.scalar.activation(
                        tanh_out[:, j : j + 1, :],
                        ftile[:, j : j + 1, :],
                        mybir.ActivationFunctionType.Tanh,
                        scale=dec[l][:, dc : dc + 1],
                    )

        for lp in range(L):
            ps = psum.tile([B, DA], f32, tag="ps", name="ps")
            for l in range(lp + 1):
                w = wpool.tile([P, DC, DA], f16, tag="w", name="w")
                nc.sync.dma_start(w[:], W_ap[lp][l])
                for dc in range(DC):
                    lhsT = ftile[:, l * DC + dc : l * DC + dc + 1, :]
                    start = l == 0 and dc == 0
                    stop = l == lp and dc == DC - 1
                    nc.tensor.matmul(
                        ps[:, 0:512],
                        lhsT,
                        w[:, dc : dc + 1, 0:512],
                        start=start,
                        stop=stop,
                    )
                    nc.tensor.matmul(
                        ps[:, 512:DA],
                        lhsT,
                        w[:, dc : dc + 1, 512:DA],
                        start=start,
                        stop=stop,
                    )
                    emit_norm(w, l, dc, lp)
                if lp == L - 1 and l in (L // 2 - 1, L - 1):
                    finalize_chunk(
                        range(0, L // 2) if l == L // 2 - 1 else range(L // 2, L)
                    )
            stage = outp.tile([B, DA], f32, tag="stage", name="stage")
            nc.vector.tensor_copy(stage[:], ps[:])
            nc.sync.dma_start(recons_h[:, lp : lp + 1, :], stage[:])

        tanh_tot = const.tile([P, 1], f32, tag="tanhtot", name="tanhtot")
        nc.vector.tensor_reduce(
            tanh_tot[:],
            tanh_out[:],
            axis=mybir.AxisListType.XY,
            op=mybir.AluOpType.add,
        )
        nc.sync.dma_start(tanh_part_h[:], tanh_tot[:])

    _split_multi_waits(nc)
    return nc


def prep_inputs(inputs):
    """Shard + cast + transpose the full inputs into per-core input maps."""
    import ml_dtypes

    wdt = np.float16 if W_DTYPE == "float16" else ml_dtypes.bfloat16
    feat = np.ascontiguousarray(np.asarray(inputs["features"]), dtype=np.float32)
    # d = core*768 + dc*128 + p
    ft = feat.reshape(B, L, NCORES, DC, P).transpose(2, 4, 1, 3, 0)  # core,p,l,dc,b
    ft = np.ascontiguousarray(ft).astype(wdt).reshape(NCORES, P, L * DC, B)

    in_maps = [{"featT": ft[i]} for i in range(NCORES)]
    for lp in range(L):
        Wl = np.asarray(inputs[f"W_{lp}"])  # [lp+1, 6144, 768]
        Ws = (
            Wl.reshape(lp + 1, NCORES, DL, DA)
            .transpose(1, 0, 2, 3)
            .astype(wdt)
            .reshape(NCORES, (lp + 1) * DL, DA)
        )
        Ws = np.ascontiguousarray(Ws)
        for i in range(NCORES):
            in_maps[i][f"W_{lp}"] = Ws[i]
    return in_maps


_NC_CACHE = {}


def get_nc():
    if "nc" not in _NC_CACHE:
        _NC_CACHE["nc"] = build_nc()
    return _NC_CACHE["nc"]


def run(inputs, trace=False, **kw):
    from concourse.bass_utils import run_bass_kernel_spmd

    nc = get_nc()
    in_maps = prep_inputs(inputs)
    res = run_bass_kernel_spmd(
        nc, in_maps, core_ids=list(range(NCORES)), trace=trace, **kw
    )
    recons = np.sum(
        np.stack([r["recons"] for r in res.results]), axis=0, dtype=np.float64
    ).astype(np.float32)
    tanh_total = float(
        np.sum([np.sum(r["tanh_part"], dtype=np.float64) for r in res.results])
    )
    sparsity = np.float32(LAMBDA / B * tanh_total)
    return (recons, sparsity), res


def kernel(**inputs):
    out, _ = run(inputs, trace=False)
    return out


# revision 13
# speedup vs baseline: 1.2684x; 1.0135x over previous
"""Trainium2 Bass kernel for nn_Decoder (ragged per-layer decoder + norms/tanh/sparsity).

Self-contained: takes FULL inputs, shards d_features 8-ways across NeuronCores,
runs one SPMD Bass/Tile kernel, and reduces partial outputs on host.

reference semantics (B=32, L=12, D=6144, A=768, C=0.1, LAMBDA=0.1):
  recons[b, lp, a]   = sum_{l<=lp, d} feat[b, l, d] * W_lp[l, d, a]
  dec_norms[l, d]    = sum_{lp>=l} ||W_lp[l, d, :]||_2
  tanh               = tanh(feat * dec_norms * C)
  sparsity           = LAMBDA * sum_d mean_b sum_l tanh

Sharding: d-axis split 8 ways (768 per core). Each core computes partial
recons (summed over its d-shard; host adds the 8 partials) and a partial
tanh-sum vector (host finishes the scalar). Weights are cast to fp16 on host
(halves the HBM traffic, which is the roofline: weights are ~1.5 GB used
exactly once); matmul accumulates in fp32 PSUM. Norms are computed in fp32
from the fp16 tiles.
"""

import sys

if "/opt/trn_rl_repo" not in sys.path:
    sys.path.insert(0, "/opt/trn_rl_repo")

import numpy as np

B = 32
L = 12
DFULL = 6144
DA = 768
NCORES = 8
DL = DFULL // NCORES  # 768 d-features per core
DC = DL // 128  # 6 chunks of 128 partitions
P = 128
C = 0.1
LAMBDA = 0.1

W_BUFS = 16  # weight-slab prefetch depth ([128, 6, 768] fp16 = 9 KiB/partition each)
# norm-op engine split pattern across DVE / ACT / GPSIMD (by per-tile op cost)
NORM_PATTERN = ("v", "a")  # alternate DVE / ACT (GPSIMD can't run STT in this walrus)
SCALE_SQ = 1024.0  # squares scaled so the fp16 scratch stays in normal range
W_DTYPE = "float16"  # "float16" (safer) or "bfloat16" (may unlock 2x DVE mode)


def _split_multi_waits(nc, maxw: int = 1):
    """This container's walrus rejects more than one semaphore wait per
    instruction ("Too many sync wait commands"), while Tile freely attaches
    several. Move excess waits onto preceding same-engine NOPs."""
    from concourse import mybir

    ctr = 0
    for bb in nc.main_func.blocks:
        out = []
        changed = False
        for ins in bb.instructions:
            si = getattr(ins, "sync_info", None)
            waits = list(si.on_wait) if si is not None and si.on_wait else []
            if len(waits) > maxw:
                changed = True
                extra, keep = waits[:-maxw], waits[-maxw:]
                for i in range(0, len(extra), maxw):
                    ctr += 1
                    nop = mybir.InstNoOp(name=f"waitsplit-{ctr}", ins=[], outs=[])
                    nop.engine = ins.engine
                    nop.sync_info = mybir.SyncInfo(
                        on_wait=extra[i : i + maxw], on_update=[]
                    )
                    out.append(nop)
                ins.sync_info = mybir.SyncInfo(
                    on_wait=keep, on_update=list(si.on_update or [])
                )
            out.append(ins)
        if changed:
            bb.instructions = out


def build_nc():
    """Build the per-core Bass module (same program on all 8 cores)."""
    import concourse.bacc as bacc
    import concourse.tile as tile
    from concourse import mybir

    f16 = getattr(mybir.dt, W_DTYPE)
    f32 = mybir.dt.float32
    bf16 = mybir.dt.bfloat16

    nc = bacc.Bacc("TRN2", target_bir_lowering=False)

    featT_h = nc.dram_tensor("featT", [P, L * DC, B], f16, kind="ExternalInput")
    W_h = [
        nc.dram_tensor(f"W_{lp}", [(lp + 1) * DL, DA], f16, kind="ExternalInput")
        for lp in range(L)
    ]
    recons_h = nc.dram_tensor("recons", [B, L, DA], f32, kind="ExternalOutput")
    tanh_part_h = nc.dram_tensor("tanh_part", [P, 1], f32, kind="ExternalOutput")

    # DRAM view of W_lp with row = l*768 + dc*128 + p  ->  [l, p, dc, a]
    W_ap = [
        W_h[lp][:].rearrange("(l dc p) a -> l p dc a", dc=DC, p=P) for lp in range(L)
    ]

    from contextlib import ExitStack

    with tile.TileContext(nc) as tc, ExitStack() as ctx:
        wpool = ctx.enter_context(tc.tile_pool(name="w", bufs=W_BUFS))
        const = ctx.enter_context(tc.tile_pool(name="const", bufs=1))
        sqpool = ctx.enter_context(tc.tile_pool(name="sq", bufs=6))
        rpool = ctx.enter_context(tc.tile_pool(name="r", bufs=6))
        psum = ctx.enter_context(tc.tile_pool(name="ps", bufs=3, space="PSUM"))
        outp = ctx.enter_context(tc.tile_pool(name="outp", bufs=3))

        ftile = const.tile([P, L * DC, B], f16, tag="ftile", name="ftile")
        nc.sync.dma_start(ftile[:], featT_h[:])

        # per-source-layer norm-square accumulators, one per engine so DVE-
        # and ACT-issued norm ops never touch the same tile. Column = dc*12+lp.
        norm_d = [const.tile([P, DC * L], f32, tag=f"nd{l}", name=f"nd{l}") for l in range(L)]
        norm_a = [const.tile([P, DC * L], f32, tag=f"na{l}", name=f"na{l}") for l in range(L)]
        for t in norm_d + norm_a:
            nc.gpsimd.memset(t[:], 0.0)

        tanh_out = const.tile([P, L * DC, B], f32, tag="tanhout", name="tanhout")
        sq_d = const.tile([P, DA], f16, tag="sqd", name="sqd")  # DVE square scratch (never read)
        sq_a = const.tile([P, DA], f16, tag="sqa", name="sqa")  # ACT square scratch (never read)
        dec = [const.tile([P, DC], f32, tag=f"dec{l}", name=f"dec{l}") for l in range(L)]

        norm_ctr = 0

        def emit_norm(wtile, l, dc, lp):
            nonlocal norm_ctr
            col = dc * L + lp
            eng = NORM_PATTERN[norm_ctr % len(NORM_PATTERN)]
            norm_ctr += 1
            if lp == L - 1 and l > 2:
                # keep ACT free for the finalize sqrt/tanh batches at the tail
                eng = "v"
            wsl = wtile[:, dc : dc + 1, :]
            if eng == "a":
                # out = (w*32)^2 = SCALE_SQ*w^2 ; accum_out = row-sum
                nc.scalar.activation(
                    sq_a[:],
                    wsl,
                    mybir.ActivationFunctionType.Square,
                    scale=float(np.sqrt(SCALE_SQ)),
                    accum_out=norm_a[l][:, col : col + 1],
                )
            else:
                # out = (w * SCALE_SQ) * w ; accum_out = row-sum = SCALE_SQ*||row||^2
                nc.vector.scalar_tensor_tensor(
                    out=sq_d[:],
                    in0=wsl,
                    scalar=SCALE_SQ,
                    in1=wsl,
                    op0=mybir.AluOpType.mult,
                    op1=mybir.AluOpType.mult,
                    accum_out=norm_d[l][:, col : col + 1],
                )

        def finalize_chunk(ls):
            """For each l in ls: dec[l] = C * sum_lp sqrt(norm_sq); then the
            tanh tiles. Batched so the ACT function table switches only twice
            per chunk (Sqrt once, Tanh once) instead of per-l."""
            rs = {}
            for l in ls:
                s_d = sqpool.tile([P, DC * L], f32, tag="s3", name="s3")
                s_a = sqpool.tile([P, DC * L], f32, tag="s3", name="s3")
                # sqrt(x * C^2 / SCALE_SQ) = C * sqrt(||row||^2)
                for srcb, dst in ((norm_d[l], s_d), (norm_a[l], s_a)):
                    nc.scalar.activation(
                        dst[:],
                        srcb[:],
                        mybir.ActivationFunctionType.Sqrt,
                        scale=C * C / SCALE_SQ,
                    )
                rs[l] = (s_d, s_a)
            for l in ls:
                s_d, s_a = rs[l]
                r_d = rpool.tile([P, DC], f32, tag="r", name="r")
                r_a = rpool.tile([P, DC], f32, tag="r", name="r")
                for s, r in ((s_d, r_d), (s_a, r_a)):
                    nc.vector.tensor_reduce(
                        r[:],
                        s[:].rearrange("p (dc lp) -> p dc lp", lp=L),
                        axis=mybir.AxisListType.X,
                        op=mybir.AluOpType.add,
                    )
                nc.vector.tensor_add(dec[l][:], r_d[:], r_a[:])
            for l in ls:
                for dc in range(DC):
                    j = l * DC + dc
                    nc.scalar.activation(
                        tanh_out[:, j : j + 1, :],
                        ftile[:, j : j + 1, :],
                        mybir.ActivationFunctionType.Tanh,
                        scale=dec[l][:, dc : dc + 1],
                    )

        for lp in range(L):
            ps = psum.tile([B, DA], f32, tag="ps", name="ps")
            for l in range(lp + 1):
                w = wpool.tile([P, DC, DA], f16, tag="w", name="w")
                nc.sync.dma_start(w[:], W_ap[lp][l])
                for dc in range(DC):
                    lhsT = ftile[:, l * DC + dc : l * DC + dc + 1, :]
                    start = l == 0 and dc == 0
                    stop = l == lp and dc == DC - 1
                    nc.tensor.matmul(
                        ps[:, 0:512],
                        lhsT,
                        w[:, dc : dc + 1, 0:512],
                        start=start,
                        stop=stop,
                    )
                    nc.tensor.matmul(
                        ps[:, 512:DA],
                        lhsT,
                        w[:, dc : dc + 1, 512:DA],
                        start=start,
                        stop=stop,
                    )
                    emit_norm(w, l, dc, lp)
                if lp == L - 1 and l in (L // 2 - 1, L - 1):
                    finalize_chunk(
                        range(0, L // 2) if l == L // 2 - 1 else range(L // 2, L)
                    )
            stage = outp.tile([B, DA], f32, tag="stage", name="stage")
            nc.vector.tensor_copy(stage[:], ps[:])
            nc.sync.dma_start(recons_h[:, lp : lp + 1, :], stage[:])

        tanh_tot = const.tile([P, 1], f32, tag="tanhtot", name="tanhtot")
        nc.vector.tensor_reduce(
            tanh_tot[:],
            tanh_out[:],
            axis=mybir.AxisListType.XY,
            op=mybir.AluOpType.add,
        )
        nc.sync.dma_start(tanh_part_h[:], tanh_tot[:])

    nc.finalize()
    return nc


def prep_inputs(inputs):
    """Shard + cast + transpose the full inputs into per-core input maps."""
    import ml_dtypes

    wdt = np.float16 if W_DTYPE == "float16" else ml_dtypes.bfloat16
    feat = np.ascontiguousarray(np.asarray(inputs["features"]), dtype=np.float32)
    # d = core*768 + dc*128 + p
    ft = feat.reshape(B, L, NCORES, DC, P).transpose(2, 4, 1, 3, 0)  # core,p,l,dc,b
    ft = np.ascontiguousarray(ft).astype(wdt).reshape(NCORES, P, L * DC, B)

    in_maps = [{"featT": ft[i]} for i in range(NCORES)]
    for lp in range(L):
        Wl = np.asarray(inputs[f"W_{lp}"])  # [lp+1, 6144, 768]
        Ws = (
            Wl.reshape(lp + 1, NCORES, DL, DA)
            .transpose(1, 0, 2, 3)
            .astype(wdt)
            .reshape(NCORES, (lp + 1) * DL, DA)
        )
        Ws = np.ascontiguousarray(Ws)
        for i in range(NCORES):
            in_maps[i][f"W_{lp}"] = Ws[i]
    return in_maps


_NC_CACHE = {}


def get_nc():
    if "nc" not in _NC_CACHE:
        _NC_CACHE["nc"] = build_nc()
    return _NC_CACHE["nc"]


def run(inputs, trace=False, **kw):
    from concourse.bass_utils import run_bass_kernel_spmd

    nc = get_nc()
    in_maps = prep_inputs(inputs)
    res = run_bass_kernel_spmd(
        nc, in_maps, core_ids=list(range(NCORES)), trace=trace, **kw
    )
    recons = np.sum(
        np.stack([r["recons"] for r in res.results]), axis=0, dtype=np.float64
    ).astype(np.float32)
    tanh_total = float(
        np.sum([np.sum(r["tanh_part"], dtype=np.float64) for r in res.results])
    )
    sparsity = np.float32(LAMBDA / B * tanh_total)
    return (recons, sparsity), res


def kernel(**inputs):
    out, _ = run(inputs, trace=False)
    return out
